# revision 2
# baseline (speedup 1.0000x reference)
"""CorrGame GDA update kernel for 8x TRN2 NeuronCores (Bass/Tile).

Computes, given X[128,1M], Y[32,1M], W[32,128], M[32,32]:
    Y_new = Y + 0.01*(W@X - M@Y)
    W_new = W + 0.1*((Y@X.T)/T - W)
    M_new = M + 0.05*0.5*((Y@Y.T)/T - M)

Sharding: T split across 8 cores. Each core computes its Y_new slice plus
partial correlation sums; host does the (tiny) final reduction.

Device-side math (per core, per 512-column tile):
    psum_y = (0.01*W.T).T @ X_bf16  +  (I - 0.01*M.T).T @ (Y_hi + Y_lo)
           = Y_new tile exactly (identity fold); Y is split hi/lo bf16 on host
             so the PE runs at full bf16 rate with ~1e-5 reconstruction error.
    Correlations contract over t, so X tiles are transposed on-device with the
    DMA xbar (bf16) in 128-column chunks; Y.T comes pre-transposed from host.
    Per chunk: psum_c[strip, :128] += Y_chunk @ X_chunk.T (lhsT = Y.T chunk),
               psum_c[strip, 128:160] += Y_chunk @ Y_chunk.T,
    with 4 col-strips of the PE array used round-robin so chunks overlap.
"""
import sys

for _p in ("/opt/trn_rl_repo",):
    if _p not in sys.path:
        sys.path.insert(0, _p)

from contextlib import ExitStack

import numpy as np
import ml_dtypes

import concourse.bass as bass
import concourse.tile as tile
from concourse import bacc, mybir
from concourse.bass_utils import run_bass_kernel_spmd

BF16 = mybir.dt.bfloat16
F32 = mybir.dt.float32
bf = ml_dtypes.bfloat16

N, K, T = 128, 32, 1_000_000
ETA_Y, ETA_W, ETA_M = 0.01, 0.1, 0.05
NCORES = 8
T_CORE = T // NCORES            # 125000
SUB = 512                       # delta-Y tile width (one PSUM bank of f32)
CHUNK = 128                     # transpose chunk width
T_PAD = ((T_CORE + 2048 - 1) // 2048) * 2048  # pad so 4-subtile groups divide evenly
GROUPS = T_PAD // (4 * SUB)
N_CHUNKS = T_PAD // CHUNK

_NC_CACHE = {}


def build_nc():
    if "nc" in _NC_CACHE:
        return _NC_CACHE["nc"]
    nc = bacc.Bacc("TRN2", target_bir_lowering=False, debug=False,
                   num_devices=NCORES)

    x_d = nc.dram_tensor("x", [N, T_PAD], BF16, kind="ExternalInput").ap()
    yhi_d = nc.dram_tensor("yhi", [K, T_PAD], BF16, kind="ExternalInput").ap()
    ylo_d = nc.dram_tensor("ylo", [K, T_PAD], BF16, kind="ExternalInput").ap()
    yt_d = nc.dram_tensor("yt", [T_PAD, K], BF16, kind="ExternalInput").ap()
    wt_d = nc.dram_tensor("wt", [N, K], BF16, kind="ExternalInput").ap()
    im_d = nc.dram_tensor("im", [K, K], BF16, kind="ExternalInput").ap()

    ynew_d = nc.dram_tensor("ynew", [K, T_PAD], F32, kind="ExternalOutput").ap()
    corr_d = nc.dram_tensor("corr", [N, 160], F32, kind="ExternalOutput").ap()

    with tile.TileContext(nc) as tc, ExitStack() as ctx:
        const = ctx.enter_context(tc.tile_pool(name="const", bufs=1))
        xp = ctx.enter_context(tc.tile_pool(name="xp", bufs=3))
        yp = ctx.enter_context(tc.tile_pool(name="yp", bufs=3))
        ytp = ctx.enter_context(tc.tile_pool(name="ytp", bufs=3))
        xtp = ctx.enter_context(tc.tile_pool(name="xtp", bufs=24))
        outp = ctx.enter_context(tc.tile_pool(name="outp", bufs=3))
        psy = ctx.enter_context(tc.tile_pool(name="psy", bufs=2, space="PSUM"))
        psc = ctx.enter_context(tc.tile_pool(name="psc", bufs=1, space="PSUM"))

        wt_sb = const.tile([N, K], BF16)
        nc.sync.dma_start(wt_sb[:], wt_d[:])
        im_sb = const.tile([K, K], BF16)
        nc.sync.dma_start(im_sb[:], im_d[:])

        ps_c = psc.tile([N, 160], F32)  # held across the whole kernel

        chunk_glob = 0
        for g in range(GROUPS):
            gw = 4 * SUB  # group width in t columns
            g0 = g * gw
            nsub = 4
            nch = gw // CHUNK  # 16

            x_sb = xp.tile([N, gw], BF16, tag="x")
            nc.sync.dma_start(x_sb[:], x_d[:, g0:g0 + gw])
            yhi_sb = yp.tile([K, gw], BF16, tag="yhi")
            nc.sync.dma_start(yhi_sb[:], yhi_d[:, g0:g0 + gw])
            ylo_sb = yp.tile([K, gw], BF16, tag="ylo")
            nc.sync.dma_start(ylo_sb[:], ylo_d[:, g0:g0 + gw])
            yt_sb = ytp.tile([CHUNK, nch * K], BF16, tag="yt")
            nc.sync.dma_start(
                yt_sb[:].rearrange("p (c k) -> p c k", k=K),
                yt_d[g0:g0 + gw, :].rearrange("(c p) k -> p c k", p=CHUNK),
            )

            # delta-Y: 4 subtiles onto 4 PE col-strips, one packed psum bank
            ps_y = psy.tile([N, SUB], F32, tag="psy")
            for j in range(nsub):
                sl = slice(j * SUB, (j + 1) * SUB)
                st = slice(32 * j, 32 * (j + 1))
                tp = (0, 32 * j)
                nc.tensor.matmul(ps_y[st, :], wt_sb[:], x_sb[:, sl],
                                 start=True, stop=False, tile_position=tp)
            for j in range(nsub):
                sl = slice(j * SUB, (j + 1) * SUB)
                st = slice(32 * j, 32 * (j + 1))
                tp = (0, 32 * j)
                nc.tensor.matmul(ps_y[st, :], im_sb[:], yhi_sb[:, sl],
                                 start=False, stop=False, tile_position=tp)
                nc.tensor.matmul(ps_y[st, :], im_sb[:], ylo_sb[:, sl],
                                 start=False, stop=True, tile_position=tp)

            ynew_sb = outp.tile([N, SUB], F32, tag="ynew")
            nc.vector.tensor_copy(ynew_sb[:], ps_y[:])
            for j in range(nsub):
                nc.sync.dma_start(
                    ynew_d[:, g0 + j * SUB:g0 + (j + 1) * SUB],
                    ynew_sb[j * K:(j + 1) * K, :])

            # correlations: transpose each 128-chunk of X, accumulate
            for c in range(nch):
                xt_sb = xtp.tile([CHUNK, CHUNK], BF16, tag="xt")
                eng = nc.scalar if (c % 2 == 0) else nc.sync
                eng.dma_start(xt_sb[:], x_sb[:, c * CHUNK:(c + 1) * CHUNK],
                              transpose=True)
                s = chunk_glob % 4
                st = slice(32 * s, 32 * (s + 1))
                tp = (0, 32 * s)
                yt_c = yt_sb[:, c * K:(c + 1) * K]
                first = chunk_glob < 4
                last = chunk_glob >= N_CHUNKS - 4
                nc.tensor.matmul(ps_c[st, 0:N], yt_c, xt_sb[:],
                                 start=first, stop=last, tile_position=tp)
                nc.tensor.matmul(ps_c[st, N:160], yt_c, yt_c,
                                 start=first, stop=last, tile_position=tp)
                chunk_glob += 1

        assert chunk_glob == N_CHUNKS

        corr_sb = const.tile([N, 160], F32)
        nc.vector.tensor_copy(corr_sb[:], ps_c[:])
        nc.sync.dma_start(corr_d[:], corr_sb[:])

    nc.compile()
    _NC_CACHE["nc"] = nc
    return nc


def _prep_inputs(X, Y, W, M):
    X = np.asarray(X, dtype=np.float32)
    Y = np.asarray(Y, dtype=np.float32)
    W = np.asarray(W, dtype=np.float32)
    M = np.asarray(M, dtype=np.float32)

    wt = np.ascontiguousarray((ETA_Y * W.T)).astype(bf)
    im = np.ascontiguousarray(
        np.eye(K, dtype=np.float32) - ETA_Y * M.T).astype(bf)

    in_maps = []
    for i in range(NCORES):
        sl = slice(i * T_CORE, (i + 1) * T_CORE)
        xs = np.zeros((N, T_PAD), dtype=bf)
        xs[:, :T_CORE] = X[:, sl].astype(bf)
        ys = Y[:, sl]
        yhi = np.zeros((K, T_PAD), dtype=bf)
        yhi[:, :T_CORE] = ys.astype(bf)
        ylo = np.zeros((K, T_PAD), dtype=bf)
        ylo[:, :T_CORE] = (ys - yhi[:, :T_CORE].astype(np.float32)).astype(bf)
        yt = np.zeros((T_PAD, K), dtype=bf)
        yt[:T_CORE, :] = ys.T.astype(bf)
        in_maps.append({"x": xs, "yhi": yhi, "ylo": ylo,
                        "yt": np.ascontiguousarray(yt), "wt": wt, "im": im})
    return in_maps


def _assemble(results, Y, W, M):
    W = np.asarray(W, dtype=np.float32)
    M = np.asarray(M, dtype=np.float32)
    ynew = np.concatenate(
        [r["ynew"][:, :T_CORE] for r in results], axis=1)

    corr = np.zeros((N, 160), dtype=np.float64)
    for r in results:
        corr += r["corr"].astype(np.float64)
    cyx = sum(corr[32 * j:32 * (j + 1), 0:N] for j in range(4))   # [32, 128]
    cyy = sum(corr[32 * j:32 * (j + 1), N:160] for j in range(4))  # [32, 32]

    t = float(T)
    w_new = (W + ETA_W * (cyx / t - W)).astype(np.float32)
    m_new = (M + ETA_M * 0.5 * (cyy / t - M)).astype(np.float32)
    return ynew, w_new, m_new


def run(X, Y, W, M, trace=False):
    nc = build_nc()
    in_maps = _prep_inputs(X, Y, W, M)
    res = run_bass_kernel_spmd(nc, in_maps, list(range(NCORES)), trace=trace)
    out = _assemble(res.results, Y, W, M)
    return out, res


def kernel(X, Y, W, M):
    out, _ = run(X, Y, W, M, trace=False)
    return out


# revision 4
# speedup vs baseline: 2.8909x; 2.8909x over previous
"""CorrGame GDA update kernel for 8x TRN2 NeuronCores (Bass/Tile).

Computes, given X[128,1M], Y[32,1M], W[32,128], M[32,32]:
    Y_new = Y + 0.01*(W@X - M@Y)
    W_new = W + 0.1*((Y@X.T)/T - W)
    M_new = M + 0.05*0.5*((Y@Y.T)/T - M)

Sharding: T split across 8 cores; each core computes its Y_new slice plus
partial correlation sums; host does the (tiny) final reduction.

Device-side per core, per 512-column subtile:
    psum_y = (0.01*W.T).T @ X_bf16 + ([IM;IM]).T @ [Y_hi;Y_lo]  (= Y_new tile,
    identity fold, IM = I - 0.01*M.T; Y split hi/lo bf16 on host so the PE
    runs full bf16 rate with ~1e-5 reconstruction error).
    Correlations contract over t: X is transposed on-device by the DMA xbar
    (one multi-chunk transpose per 4096 cols), Y.T comes from host. Per
    128-col chunk one matmul with stationary Y.T_c and moving [X.T_c | Y.T_c]
    accumulates Y@X.T and Y@Y.T partials into a held PSUM bank, rotating over
    4 PE column strips.
"""
import sys

for _p in ("/opt/trn_rl_repo",):
    if _p not in sys.path:
        sys.path.insert(0, _p)

from contextlib import ExitStack

import numpy as np
import ml_dtypes

import concourse.bass as bass
import concourse.tile as tile
from concourse import bacc, mybir
from concourse.bass_utils import run_bass_kernel_spmd

BF16 = mybir.dt.bfloat16
F32 = mybir.dt.float32
bf = ml_dtypes.bfloat16

N, K, T = 128, 32, 1_000_000
ETA_Y, ETA_W, ETA_M = 0.01, 0.1, 0.05
NCORES = 8
T_CORE = T // NCORES            # 125000
SUB = 512                       # delta-Y tile width (one PSUM bank of f32)
CHUNK = 128                     # transpose chunk width
SG = 4096                       # super-group width (batched DMA granularity)
T_PAD = ((T_CORE + SG - 1) // SG) * SG  # 126976
NSG = T_PAD // SG               # 31
N_CHUNKS = T_PAD // CHUNK       # 992
CW = CHUNK + K                  # 160: combined [X.T_c | Y.T_c] width

_NC_CACHE = {}


def build_nc():
    if "nc" in _NC_CACHE:
        return _NC_CACHE["nc"]
    nc = bacc.Bacc("TRN2", target_bir_lowering=False, debug=False,
                   num_devices=NCORES)

    x_d = nc.dram_tensor("x", [N, T_PAD], BF16, kind="ExternalInput").ap()
    yhl_d = nc.dram_tensor("yhl", [2 * K, T_PAD], BF16, kind="ExternalInput").ap()
    yt_d = nc.dram_tensor("yt", [T_PAD, K], BF16, kind="ExternalInput").ap()
    wt_d = nc.dram_tensor("wt", [N, K], BF16, kind="ExternalInput").ap()
    imm_d = nc.dram_tensor("imm", [2 * K, K], BF16, kind="ExternalInput").ap()

    ynew_d = nc.dram_tensor("ynew", [K, T_PAD], F32, kind="ExternalOutput").ap()
    corr_d = nc.dram_tensor("corr", [N, CW], F32, kind="ExternalOutput").ap()

    nch = SG // CHUNK   # 32 chunks per super-group
    ngr = SG // (4 * SUB)  # 2 psum-groups per super-group

    with tile.TileContext(nc) as tc, ExitStack() as ctx:
        const = ctx.enter_context(tc.tile_pool(name="const", bufs=1))
        xp = ctx.enter_context(tc.tile_pool(name="xp", bufs=2))
        yp = ctx.enter_context(tc.tile_pool(name="yp", bufs=2))
        ctp = ctx.enter_context(tc.tile_pool(name="ctp", bufs=2))
        outp = ctx.enter_context(tc.tile_pool(name="outp", bufs=2))
        psy = ctx.enter_context(tc.tile_pool(name="psy", bufs=2, space="PSUM"))
        psc = ctx.enter_context(tc.tile_pool(name="psc", bufs=1, space="PSUM"))

        wt_sb = const.tile([N, K], BF16)
        nc.sync.dma_start(wt_sb[:], wt_d[:])
        imm_sb = const.tile([2 * K, K], BF16)
        nc.sync.dma_start(imm_sb[:], imm_d[:])

        ps_c = psc.tile([N, CW], F32)  # held across the whole kernel

        chunk_glob = 0
        for g in range(NSG):
            g0 = g * SG

            x_sb = xp.tile([N, SG], BF16, tag="x")
            nc.sync.dma_start(x_sb[:], x_d[:, g0:g0 + SG])
            yhl_sb = yp.tile([2 * K, SG], BF16, tag="yhl")
            nc.sync.dma_start(yhl_sb[:], yhl_d[:, g0:g0 + SG])

            # combined [X.T_c | Y.T_c] tile: chunk c occupies cols [c*CW, c*CW+CW)
            ct_sb = ctp.tile([CHUNK, nch * CW], BF16, tag="ct")
            nc.scalar.dma_start(
                ct_sb[:].rearrange("p (c w) -> p c w", w=CW)[:, :, 0:CHUNK],
                x_sb[:],
                transpose=True,
            )
            nc.scalar.dma_start(
                ct_sb[:].rearrange("p (c w) -> p c w", w=CW)[:, :, CHUNK:CW],
                yt_d[g0:g0 + SG, :].rearrange("(c p) k -> p c k", p=CHUNK),
            )

            stage_sb = outp.tile([N, ngr * SUB], F32, tag="stage")

            for h in range(ngr):
                h0 = h * 4 * SUB
                ps_y = psy.tile([N, SUB], F32, tag="psy")
                for j in range(4):
                    sl = slice(h0 + j * SUB, h0 + (j + 1) * SUB)
                    st = slice(32 * j, 32 * (j + 1))
                    tp = (0, 32 * j)
                    nc.tensor.matmul(ps_y[st, :], wt_sb[:], x_sb[:, sl],
                                     start=True, stop=False, tile_position=tp)
                    nc.tensor.matmul(ps_y[st, :], imm_sb[:], yhl_sb[:, sl],
                                     start=False, stop=True, tile_position=tp)
                nc.vector.tensor_copy(
                    stage_sb[:, h * SUB:(h + 1) * SUB], ps_y[:])

            # correlations over this super-group's 32 chunks
            for c in range(nch):
                s = chunk_glob % 4
                st = slice(32 * s, 32 * (s + 1))
                tp = (0, 32 * s)
                yt_c = ct_sb[:, c * CW + CHUNK:(c + 1) * CW]
                mv = ct_sb[:, c * CW:(c + 1) * CW]
                first = chunk_glob < 4
                last = chunk_glob >= N_CHUNKS - 4
                nc.tensor.matmul(ps_c[st, :], yt_c, mv,
                                 start=first, stop=last, tile_position=tp)
                chunk_glob += 1

            # store Y_new for this super-group: 4 DMAs (one per strip)
            for j in range(4):
                nc.gpsimd.dma_start(
                    ynew_d[:, g0:g0 + SG]
                    .rearrange("k (h j q) -> j k h q", j=4, q=SUB)[j],
                    stage_sb[j * K:(j + 1) * K, :]
                    .rearrange("k (h q) -> k h q", q=SUB),
                )

        assert chunk_glob == N_CHUNKS

        corr_sb = const.tile([N, CW], F32)
        nc.vector.tensor_copy(corr_sb[:], ps_c[:])
        nc.sync.dma_start(corr_d[:], corr_sb[:])

    nc.compile()
    _NC_CACHE["nc"] = nc
    return nc


def _prep_inputs(X, Y, W, M):
    X = np.asarray(X, dtype=np.float32)
    Y = np.asarray(Y, dtype=np.float32)
    W = np.asarray(W, dtype=np.float32)
    M = np.asarray(M, dtype=np.float32)

    wt = np.ascontiguousarray(ETA_Y * W.T).astype(bf)
    im = (np.eye(K, dtype=np.float32) - ETA_Y * M.T).astype(bf)
    imm = np.ascontiguousarray(np.concatenate([im, im], axis=0))  # [64, 32]

    in_maps = []
    for i in range(NCORES):
        sl = slice(i * T_CORE, (i + 1) * T_CORE)
        xs = np.zeros((N, T_PAD), dtype=bf)
        xs[:, :T_CORE] = X[:, sl].astype(bf)
        ys = Y[:, sl]
        yhl = np.zeros((2 * K, T_PAD), dtype=bf)
        yhl[:K, :T_CORE] = ys.astype(bf)
        yhl[K:, :T_CORE] = (ys - yhl[:K, :T_CORE].astype(np.float32)).astype(bf)
        yt = np.zeros((T_PAD, K), dtype=bf)
        yt[:T_CORE, :] = ys.T.astype(bf)
        in_maps.append({"x": xs, "yhl": yhl,
                        "yt": np.ascontiguousarray(yt), "wt": wt, "imm": imm})
    return in_maps


def _assemble(results, Y, W, M):
    W = np.asarray(W, dtype=np.float32)
    M = np.asarray(M, dtype=np.float32)
    ynew = np.concatenate(
        [r["ynew"][:, :T_CORE] for r in results], axis=1)

    corr = np.zeros((N, CW), dtype=np.float64)
    for r in results:
        corr += r["corr"].astype(np.float64)
    cyx = sum(corr[32 * j:32 * (j + 1), 0:N] for j in range(4))    # [32, 128]
    cyy = sum(corr[32 * j:32 * (j + 1), N:CW] for j in range(4))   # [32, 32]

    t = float(T)
    w_new = (W + ETA_W * (cyx / t - W)).astype(np.float32)
    m_new = (M + ETA_M * 0.5 * (cyy / t - M)).astype(np.float32)
    return ynew, w_new, m_new


def run(X, Y, W, M, trace=False):
    nc = build_nc()
    in_maps = _prep_inputs(X, Y, W, M)
    res = run_bass_kernel_spmd(nc, in_maps, list(range(NCORES)), trace=trace)
    out = _assemble(res.results, Y, W, M)
    return out, res


def kernel(X, Y, W, M):
    out, _ = run(X, Y, W, M, trace=False)
    return out


# revision 5
# speedup vs baseline: 3.0329x; 1.0491x over previous
"""CorrGame GDA update kernel for 8x TRN2 NeuronCores (Bass/Tile).

Computes, given X[128,1M], Y[32,1M], W[32,128], M[32,32]:
    Y_new = Y + 0.01*(W@X - M@Y)
    W_new = W + 0.1*((Y@X.T)/T - W)
    M_new = M + 0.05*0.5*((Y@Y.T)/T - M)

Sharding: T split across 8 cores; each core computes its Y_new slice plus
partial correlation sums; host does the (tiny) final reduction.

Device-side per core, per 512-column subtile:
    psum_y = (0.01*W.T).T @ X_bf16 + ([IM;IM]).T @ [Y_hi;Y_lo]  (= Y_new tile,
    identity fold, IM = I - 0.01*M.T; Y split hi/lo bf16 on host so the PE
    runs full bf16 rate with ~1e-5 reconstruction error).
    Correlations contract over t: X is transposed on-device by the DMA xbar
    (one multi-chunk transpose per 4096 cols), Y.T comes from host. Per
    128-col chunk one matmul with stationary Y.T_c and moving [X.T_c | Y.T_c]
    accumulates Y@X.T and Y@Y.T partials into a held PSUM bank, rotating over
    4 PE column strips.
"""
import sys

for _p in ("/opt/trn_rl_repo",):
    if _p not in sys.path:
        sys.path.insert(0, _p)

from contextlib import ExitStack

import numpy as np
import ml_dtypes

import concourse.bass as bass
import concourse.tile as tile
from concourse import bacc, mybir
from concourse.bass_utils import run_bass_kernel_spmd

BF16 = mybir.dt.bfloat16
F32 = mybir.dt.float32
bf = ml_dtypes.bfloat16

N, K, T = 128, 32, 1_000_000
ETA_Y, ETA_W, ETA_M = 0.01, 0.1, 0.05
NCORES = 8
T_CORE = T // NCORES            # 125000
SUB = 512                       # delta-Y tile width (one PSUM bank of f32)
CHUNK = 128                     # transpose chunk width
SG = 4096                       # super-group width (batched DMA granularity)
T_PAD = ((T_CORE + SG - 1) // SG) * SG  # 126976
NSG = T_PAD // SG               # 31
N_CHUNKS = T_PAD // CHUNK       # 992
CW = CHUNK + K                  # 160: combined [X.T_c | Y.T_c] width

_NC_CACHE = {}


def build_nc():
    if "nc" in _NC_CACHE:
        return _NC_CACHE["nc"]
    nc = bacc.Bacc("TRN2", target_bir_lowering=False, debug=False,
                   num_devices=NCORES)

    x_d = nc.dram_tensor("x", [N, T_PAD], BF16, kind="ExternalInput").ap()
    yhl_d = nc.dram_tensor("yhl", [2 * K, T_PAD], BF16, kind="ExternalInput").ap()
    yt_d = nc.dram_tensor("yt", [T_PAD, K], BF16, kind="ExternalInput").ap()
    wt_d = nc.dram_tensor("wt", [N, K], BF16, kind="ExternalInput").ap()
    imm_d = nc.dram_tensor("imm", [2 * K, K], BF16, kind="ExternalInput").ap()

    ynew_d = nc.dram_tensor("ynew", [K, T_PAD], F32, kind="ExternalOutput").ap()
    corr_d = nc.dram_tensor("corr", [N, CW], F32, kind="ExternalOutput").ap()

    nch = SG // CHUNK   # 32 chunks per super-group
    ngr = SG // (4 * SUB)  # 2 psum-groups per super-group

    with tile.TileContext(nc) as tc, ExitStack() as ctx:
        const = ctx.enter_context(tc.tile_pool(name="const", bufs=1))
        xp = ctx.enter_context(tc.tile_pool(name="xp", bufs=3))
        yp = ctx.enter_context(tc.tile_pool(name="yp", bufs=3))
        ctp = ctx.enter_context(tc.tile_pool(name="ctp", bufs=4))
        outp = ctx.enter_context(tc.tile_pool(name="outp", bufs=3))
        psy = ctx.enter_context(tc.tile_pool(name="psy", bufs=3, space="PSUM"))
        psc = ctx.enter_context(tc.tile_pool(name="psc", bufs=1, space="PSUM"))

        wt_sb = const.tile([N, K], BF16)
        nc.sync.dma_start(wt_sb[:], wt_d[:])
        imm_sb = const.tile([2 * K, K], BF16)
        nc.sync.dma_start(imm_sb[:], imm_d[:])

        ps_c = psc.tile([N, CW], F32)  # held across the whole kernel

        chunk_glob = 0
        for g in range(NSG):
            g0 = g * SG

            x_sb = xp.tile([N, SG], BF16, tag="x")
            nc.sync.dma_start(x_sb[:], x_d[:, g0:g0 + SG])
            yhl_sb = yp.tile([2 * K, SG], BF16, tag="yhl")
            nc.sync.dma_start(yhl_sb[:], yhl_d[:, g0:g0 + SG])

            # combined [X.T_c | Y.T_c] tile: chunk c occupies cols [c*CW, c*CW+CW)
            ct_sb = ctp.tile([CHUNK, nch * CW], BF16, tag="ct")
            nc.scalar.dma_start(
                ct_sb[:].rearrange("p (c w) -> p c w", w=CW)[:, :, 0:CHUNK],
                x_sb[:],
                transpose=True,
            )
            nc.sync.dma_start(
                ct_sb[:].rearrange("p (c w) -> p c w", w=CW)[:, :, CHUNK:CW],
                yt_d[g0:g0 + SG, :].rearrange("(c p) k -> p c k", p=CHUNK),
            )

            stage_sb = outp.tile([N, ngr * SUB], F32, tag="stage")

            for h in range(ngr):
                h0 = h * 4 * SUB
                ps_y = psy.tile([N, SUB], F32, tag="psy")
                for j in range(4):
                    sl = slice(h0 + j * SUB, h0 + (j + 1) * SUB)
                    st = slice(32 * j, 32 * (j + 1))
                    nc.tensor.matmul(ps_y[st, :], wt_sb[:], x_sb[:, sl],
                                     start=True, stop=False,
                                     tile_position=(0, 32 * j))
                for j in range(4):
                    sl = slice(h0 + j * SUB, h0 + (j + 1) * SUB)
                    st = slice(32 * j, 32 * (j + 1))
                    nc.tensor.matmul(ps_y[st, :], imm_sb[:], yhl_sb[:, sl],
                                     start=False, stop=True,
                                     tile_position=(0, 32 * j))
                nc.vector.tensor_copy(
                    stage_sb[:, h * SUB:(h + 1) * SUB], ps_y[:])

            # correlations over this super-group's 32 chunks
            for c in range(nch):
                s = chunk_glob % 4
                st = slice(32 * s, 32 * (s + 1))
                tp = (0, 32 * s)
                yt_c = ct_sb[:, c * CW + CHUNK:(c + 1) * CW]
                mv = ct_sb[:, c * CW:(c + 1) * CW]
                first = chunk_glob < 4
                last = chunk_glob >= N_CHUNKS - 4
                nc.tensor.matmul(ps_c[st, :], yt_c, mv,
                                 start=first, stop=last, tile_position=tp)
                chunk_glob += 1

            # store Y_new for this super-group: 4 DMAs (one per strip)
            for j in range(4):
                nc.gpsimd.dma_start(
                    ynew_d[:, g0:g0 + SG]
                    .rearrange("k (h j q) -> j k h q", j=4, q=SUB)[j],
                    stage_sb[j * K:(j + 1) * K, :]
                    .rearrange("k (h q) -> k h q", q=SUB),
                )

        assert chunk_glob == N_CHUNKS

        corr_sb = const.tile([N, CW], F32)
        nc.vector.tensor_copy(corr_sb[:], ps_c[:])
        nc.sync.dma_start(corr_d[:], corr_sb[:])

    nc.compile()
    _NC_CACHE["nc"] = nc
    return nc


def _prep_inputs(X, Y, W, M):
    X = np.asarray(X, dtype=np.float32)
    Y = np.asarray(Y, dtype=np.float32)
    W = np.asarray(W, dtype=np.float32)
    M = np.asarray(M, dtype=np.float32)

    wt = np.ascontiguousarray(ETA_Y * W.T).astype(bf)
    im = (np.eye(K, dtype=np.float32) - ETA_Y * M.T).astype(bf)
    imm = np.ascontiguousarray(np.concatenate([im, im], axis=0))  # [64, 32]

    in_maps = []
    for i in range(NCORES):
        sl = slice(i * T_CORE, (i + 1) * T_CORE)
        xs = np.zeros((N, T_PAD), dtype=bf)
        xs[:, :T_CORE] = X[:, sl].astype(bf)
        ys = Y[:, sl]
        yhl = np.zeros((2 * K, T_PAD), dtype=bf)
        yhl[:K, :T_CORE] = ys.astype(bf)
        yhl[K:, :T_CORE] = (ys - yhl[:K, :T_CORE].astype(np.float32)).astype(bf)
        yt = np.zeros((T_PAD, K), dtype=bf)
        yt[:T_CORE, :] = ys.T.astype(bf)
        in_maps.append({"x": xs, "yhl": yhl,
                        "yt": np.ascontiguousarray(yt), "wt": wt, "imm": imm})
    return in_maps


def _assemble(results, Y, W, M):
    W = np.asarray(W, dtype=np.float32)
    M = np.asarray(M, dtype=np.float32)
    ynew = np.concatenate(
        [r["ynew"][:, :T_CORE] for r in results], axis=1)

    corr = np.zeros((N, CW), dtype=np.float64)
    for r in results:
        corr += r["corr"].astype(np.float64)
    cyx = sum(corr[32 * j:32 * (j + 1), 0:N] for j in range(4))    # [32, 128]
    cyy = sum(corr[32 * j:32 * (j + 1), N:CW] for j in range(4))   # [32, 32]

    t = float(T)
    w_new = (W + ETA_W * (cyx / t - W)).astype(np.float32)
    m_new = (M + ETA_M * 0.5 * (cyy / t - M)).astype(np.float32)
    return ynew, w_new, m_new


def run(X, Y, W, M, trace=False):
    nc = build_nc()
    in_maps = _prep_inputs(X, Y, W, M)
    res = run_bass_kernel_spmd(nc, in_maps, list(range(NCORES)), trace=trace)
    out = _assemble(res.results, Y, W, M)
    return out, res


def kernel(X, Y, W, M):
    out, _ = run(X, Y, W, M, trace=False)
    return out


# revision 8
# speedup vs baseline: 4.8026x; 1.5835x over previous
"""CorrGame GDA update kernel for 8x TRN2 NeuronCores (Bass/Tile).

Computes, given X[128,1M], Y[32,1M], W[32,128], M[32,32]:
    Y_new = Y + 0.01*(W@X - M@Y)
    W_new = W + 0.1*((Y@X.T)/T - W)
    M_new = M + 0.05*0.5*((Y@Y.T)/T - M)

Sharding: T split across 8 cores; each core computes its Y_new slice plus
partial correlation sums; host does the (tiny) final reduction.

Device-side per core, per 512-column subtile:
    psum_y = (0.01*W.T).T @ X_bf16 + ([IM;IM]).T @ [Y_hi;Y_lo]  (= Y_new tile,
    identity fold, IM = I - 0.01*M.T; Y split hi/lo bf16 on host so the PE
    runs full bf16 rate with ~1e-5 reconstruction error).
    Correlations contract over t: X is transposed on-device by the DMA xbar
    (one multi-chunk transpose per 4096 cols), Y.T comes from host. Per
    128-col chunk one matmul with stationary Y.T_c and moving [X.T_c | Y.T_c]
    accumulates Y@X.T and Y@Y.T partials into a held PSUM bank, rotating over
    4 PE column strips.
"""
import sys

for _p in ("/opt/trn_rl_repo",):
    if _p not in sys.path:
        sys.path.insert(0, _p)

from contextlib import ExitStack

import numpy as np
import ml_dtypes

import concourse.bass as bass
import concourse.tile as tile
from concourse import bacc, mybir
from concourse.bass_utils import run_bass_kernel_spmd

BF16 = mybir.dt.bfloat16
F32 = mybir.dt.float32
bf = ml_dtypes.bfloat16

N, K, T = 128, 32, 1_000_000
ETA_Y, ETA_W, ETA_M = 0.01, 0.1, 0.05
NCORES = 8
T_CORE = T // NCORES            # 125000
SUB = 512                       # delta-Y tile width (one PSUM bank of f32)
CHUNK = 128                     # transpose chunk width
SG = 4096                       # super-group width (batched DMA granularity)
T_PAD = ((T_CORE + SG - 1) // SG) * SG  # 126976
NSG = T_PAD // SG               # 31
N_CHUNKS = T_PAD // CHUNK       # 992
CW = CHUNK + K                  # 160: combined [X.T_c | Y.T_c] width

_NC_CACHE = {}


def build_nc():
    if "nc" in _NC_CACHE:
        return _NC_CACHE["nc"]
    nc = bacc.Bacc("TRN2", target_bir_lowering=False, debug=False,
                   num_devices=NCORES)

    x_d = nc.dram_tensor("x", [N, T_PAD], BF16, kind="ExternalInput").ap()
    yhl_d = nc.dram_tensor("yhl", [2 * K, T_PAD], BF16, kind="ExternalInput").ap()
    yt_d = nc.dram_tensor("yt", [T_PAD, K], BF16, kind="ExternalInput").ap()
    wt_d = nc.dram_tensor("wt", [N, K], BF16, kind="ExternalInput").ap()
    imm_d = nc.dram_tensor("imm", [2 * K, K], BF16, kind="ExternalInput").ap()
    ident_d = nc.dram_tensor("ident", [N, N], BF16, kind="ExternalInput").ap()

    ynew_d = nc.dram_tensor("ynew", [K, T_PAD], F32, kind="ExternalOutput").ap()
    corr_d = nc.dram_tensor("corr", [N, CW], F32, kind="ExternalOutput").ap()

    nch = SG // CHUNK   # 32 chunks per super-group
    ngr = SG // (4 * SUB)  # 2 psum-groups per super-group

    with tile.TileContext(nc) as tc, ExitStack() as ctx:
        const = ctx.enter_context(tc.tile_pool(name="const", bufs=1))
        xp = ctx.enter_context(tc.tile_pool(name="xp", bufs=3))
        yp = ctx.enter_context(tc.tile_pool(name="yp", bufs=3))
        ctp = ctx.enter_context(tc.tile_pool(name="ctp", bufs=4))
        outp = ctx.enter_context(tc.tile_pool(name="outp", bufs=3))
        psy = ctx.enter_context(tc.tile_pool(name="psy", bufs=3, space="PSUM"))
        pst = ctx.enter_context(tc.tile_pool(name="pst", bufs=2, space="PSUM"))
        psc = ctx.enter_context(tc.tile_pool(name="psc", bufs=1, space="PSUM"))

        wt_sb = const.tile([N, K], BF16)
        nc.sync.dma_start(wt_sb[:], wt_d[:])
        ident_sb = const.tile([N, N], BF16)
        nc.sync.dma_start(ident_sb[:], ident_d[:])
        imm_sb = const.tile([2 * K, K], BF16)
        nc.sync.dma_start(imm_sb[:], imm_d[:])

        ps_c = psc.tile([N, CW], F32)  # held across the whole kernel

        chunk_glob = 0
        for g in range(NSG):
            g0 = g * SG

            x_sb = xp.tile([N, SG], BF16, tag="x")
            nc.sync.dma_start(x_sb[:], x_d[:, g0:g0 + SG])
            yhl_sb = yp.tile([2 * K, SG], BF16, tag="yhl")
            nc.sync.dma_start(yhl_sb[:], yhl_d[:, g0:g0 + SG])

            # combined [X.T_c | Y.T_c] tile: chunk c occupies cols [c*CW, c*CW+CW)
            ct_sb = ctp.tile([CHUNK, nch * CW], BF16, tag="ct")
            nc.scalar.dma_start(
                ct_sb[:].rearrange("p (c w) -> p c w", w=CW)[:, :, CHUNK:CW],
                yt_d[g0:g0 + SG, :].rearrange("(c p) k -> p c k", p=CHUNK),
            )

            stage_sb = outp.tile([N, ngr * SUB], F32, tag="stage")

            for h in range(ngr):
                h0 = h * 4 * SUB
                ps_y = psy.tile([N, SUB], F32, tag="psy")
                for j in range(4):
                    sl = slice(h0 + j * SUB, h0 + (j + 1) * SUB)
                    st = slice(32 * j, 32 * (j + 1))
                    nc.tensor.matmul(ps_y[st, :], wt_sb[:], x_sb[:, sl],
                                     start=True, stop=False,
                                     tile_position=(0, 32 * j))
                for j in range(4):
                    sl = slice(h0 + j * SUB, h0 + (j + 1) * SUB)
                    st = slice(32 * j, 32 * (j + 1))
                    nc.tensor.matmul(ps_y[st, :], imm_sb[:], yhl_sb[:, sl],
                                     start=False, stop=True,
                                     tile_position=(0, 32 * j))
                nc.vector.tensor_copy(
                    stage_sb[:, h * SUB:(h + 1) * SUB], ps_y[:])

            # transpose X chunks on the PE, 4 chunks per PSUM bank,
            # evict (cast f32->bf16) into the combined ct layout
            for p in range(nch // 4):
                ps_t = pst.tile([CHUNK, 4 * CHUNK], BF16, tag="pst")
                for c2 in range(4):
                    c = 4 * p + c2
                    nc.tensor.transpose(
                        ps_t[:, c2 * CHUNK:(c2 + 1) * CHUNK],
                        x_sb[:, c * CHUNK:(c + 1) * CHUNK],
                        ident_sb[:])
                dst = (ct_sb[:].rearrange("p (c w) -> p c w", w=CW)
                       [:, 4 * p:4 * p + 4, 0:CHUNK])
                src = ps_t[:].rearrange("p (c n) -> p c n", n=CHUNK)
                if p % 2 == 0:
                    nc.vector.tensor_copy(dst, src)
                else:
                    nc.scalar.copy(dst, src)

            # correlations over this super-group's 32 chunks
            for c in range(nch):
                s = chunk_glob % 4
                st = slice(32 * s, 32 * (s + 1))
                tp = (0, 32 * s)
                yt_c = ct_sb[:, c * CW + CHUNK:(c + 1) * CW]
                mv = ct_sb[:, c * CW:(c + 1) * CW]
                first = chunk_glob < 4
                last = chunk_glob >= N_CHUNKS - 4
                nc.tensor.matmul(ps_c[st, :], yt_c, mv,
                                 start=first, stop=last, tile_position=tp)
                chunk_glob += 1

            # store Y_new for this super-group: 4 DMAs (one per strip)
            for j in range(4):
                nc.gpsimd.dma_start(
                    ynew_d[:, g0:g0 + SG]
                    .rearrange("k (h j q) -> j k h q", j=4, q=SUB)[j],
                    stage_sb[j * K:(j + 1) * K, :]
                    .rearrange("k (h q) -> k h q", q=SUB),
                )

        assert chunk_glob == N_CHUNKS

        corr_sb = const.tile([N, CW], F32)
        nc.vector.tensor_copy(corr_sb[:], ps_c[:])
        nc.sync.dma_start(corr_d[:], corr_sb[:])

    nc.compile()
    _NC_CACHE["nc"] = nc
    return nc


def _prep_inputs(X, Y, W, M):
    X = np.asarray(X, dtype=np.float32)
    Y = np.asarray(Y, dtype=np.float32)
    W = np.asarray(W, dtype=np.float32)
    M = np.asarray(M, dtype=np.float32)

    wt = np.ascontiguousarray(ETA_Y * W.T).astype(bf)
    ident = np.ascontiguousarray(np.eye(N, dtype=np.float32)).astype(bf)
    im = (np.eye(K, dtype=np.float32) - ETA_Y * M.T).astype(bf)
    imm = np.ascontiguousarray(np.concatenate([im, im], axis=0))  # [64, 32]

    in_maps = []
    for i in range(NCORES):
        sl = slice(i * T_CORE, (i + 1) * T_CORE)
        xs = np.zeros((N, T_PAD), dtype=bf)
        xs[:, :T_CORE] = X[:, sl].astype(bf)
        ys = Y[:, sl]
        yhl = np.zeros((2 * K, T_PAD), dtype=bf)
        yhl[:K, :T_CORE] = ys.astype(bf)
        yhl[K:, :T_CORE] = (ys - yhl[:K, :T_CORE].astype(np.float32)).astype(bf)
        yt = np.zeros((T_PAD, K), dtype=bf)
        yt[:T_CORE, :] = ys.T.astype(bf)
        in_maps.append({"x": xs, "yhl": yhl, "yt": np.ascontiguousarray(yt),
                        "wt": wt, "imm": imm, "ident": ident})
    return in_maps


def _assemble(results, Y, W, M):
    W = np.asarray(W, dtype=np.float32)
    M = np.asarray(M, dtype=np.float32)
    ynew = np.concatenate(
        [r["ynew"][:, :T_CORE] for r in results], axis=1)

    corr = np.zeros((N, CW), dtype=np.float64)
    for r in results:
        corr += r["corr"].astype(np.float64)
    cyx = sum(corr[32 * j:32 * (j + 1), 0:N] for j in range(4))    # [32, 128]
    cyy = sum(corr[32 * j:32 * (j + 1), N:CW] for j in range(4))   # [32, 32]

    t = float(T)
    w_new = (W + ETA_W * (cyx / t - W)).astype(np.float32)
    m_new = (M + ETA_M * 0.5 * (cyy / t - M)).astype(np.float32)
    return ynew, w_new, m_new


def run(X, Y, W, M, trace=False):
    nc = build_nc()
    in_maps = _prep_inputs(X, Y, W, M)
    res = run_bass_kernel_spmd(nc, in_maps, list(range(NCORES)), trace=trace)
    out = _assemble(res.results, Y, W, M)
    return out, res


def kernel(X, Y, W, M):
    out, _ = run(X, Y, W, M, trace=False)
    return out
